# revision 28
# baseline (speedup 1.0000x reference)
"""Deformable Conv2d (B=4, Cin=128, Cout=256, H=W=64, K=3, s=1, p=1) on 8 trn2 cores.

Sharding: core = 2*b + h  (batch b, row-half h: rows h*32 .. h*32+31).
Per-core pipeline:
  - offset/mask 3x3 conv on PE (bf16, contiguous pre-shifted inputs),
    outputs quadrant-replicated in SBUF
  - DVE/ACT scalar pipeline -> bilinear corner coefs (bf16) + int16 gather idx
  - dma_gather from a host-built padded channels-last row-pair canvas in HBM:
    one 1KB element = 2x2 corner patch x 128 channels (bf16)
  - coef replication across partitions: stride-0 SBUF->SBUF DMA broadcast
    (+ one DVE stream_shuffle to balance engines)
  - bf16 combine (coef x corners) on DVE
  - main matmul: 9 taps x 2 Cout tiles, bf16, PSUM-accumulated -> fp32 out
"""
import numpy as np
import ml_dtypes
from contextlib import ExitStack

import concourse.bacc as bacc
import concourse.bass as bass
import concourse.mybir as mybir
import concourse.tile as tile
from concourse import library_config
from concourse.bass_utils import run_bass_kernel_spmd

B, CIN, COUT, H, W, K = 4, 128, 256, 64, 64, 3
KK = K * K
NCORES = 8
HALF = H // 2            # 32 rows per core
N = HALF * W             # 2048 output positions per core
CH = 512                 # matmul chunk size (PSUM bank limit, fp32)
NCHUNK = N // CH
PCH = 1024               # scalar-pipeline chunk size
NPCH = N // PCH
PADC = 18                # canvas padding (covers reference clip of +-16 + tap + bilinear)
HC = 100                 # canvas row-pairs  (y' = y + PADC, y in [-18, 81])
WC = 104                 # canvas cols (x' = x + PADC)
ES = 512                 # gather elem size in bf16 elements (1KB): 2x2 patch x 128ch
F32 = mybir.dt.float32
BF16 = mybir.dt.bfloat16
I16 = mybir.dt.int16
BF = ml_dtypes.bfloat16

_cache = {}

# f32->int16 convert on HW DVE rounds-to-nearest-even -> floor(t) = int(t-0.5).
# CoreSim models truncation -> floor(t) = int(t). Sim checks set this to 0.0.
FLOOR_DELTA = -0.5
# How many taps use DVE stream_shuffle for coef replication (rest use
# stride-0 SBUF->SBUF DMA broadcast). Balances DVE vs DMA engine time.
N_SHUF = 1
N_POOL = 3


def _build_program(debug=False):
    nc = bacc.Bacc("TRN2", target_bir_lowering=False, debug=False,
                   enable_asserts=False, num_devices=NCORES)
    if debug:
        dbg_conv_d = nc.dram_tensor("dbg_conv", [128, N], F32,
                                    kind="ExternalOutput")
        dbg_idx_d = nc.dram_tensor("dbg_idx", [128, N], I16, kind="ExternalOutput")
        dbg_ct_d = nc.dram_tensor("dbg_ct", [128, 4 * N], BF16,
                                  kind="ExternalOutput")
        dbg_s_d = nc.dram_tensor("dbg_s", [KK, 128, N], BF16,
                                 kind="ExternalOutput")
        dbg_g_d = nc.dram_tensor("dbg_g", [128, 4 * N], BF16,
                                 kind="ExternalOutput")
    xsh_d = nc.dram_tensor("xsh", [128, 3 * 34 * 64], BF16, kind="ExternalInput")
    canvas_d = nc.dram_tensor("canvas", [HC * WC + 1, ES // 2], BF16,
                              kind="ExternalInput")
    womT_d = nc.dram_tensor("womT", [128, KK * 128], BF16, kind="ExternalInput")
    wmnT_d = nc.dram_tensor("wmnT", [128, KK * 2 * 128], BF16, kind="ExternalInput")
    biasoff_d = nc.dram_tensor("biasoff", [128, 1], F32, kind="ExternalInput")
    biasmsk_d = nc.dram_tensor("biasmsk", [128, 1], F32, kind="ExternalInput")
    baset_d = nc.dram_tensor("baset", [128, N], F32, kind="ExternalInput")
    out_d = nc.dram_tensor("out", [2, 128, N], F32, kind="ExternalOutput")

    with tile.TileContext(nc) as tc, ExitStack() as ctx:
        cpool = ctx.enter_context(tc.tile_pool(name="const", bufs=1))
        ppool = ctx.enter_context(tc.tile_pool(name="pipe", bufs=1))
        gpool = ctx.enter_context(tc.tile_pool(name="gath", bufs=2))
        rpool = ctx.enter_context(tc.tile_pool(name="crep", bufs=2))
        spool = ctx.enter_context(tc.tile_pool(name="samp", bufs=2))
        opool = ctx.enter_context(tc.tile_pool(name="outp", bufs=2))
        dpool = ctx.enter_context(tc.tile_pool(name="dram", bufs=1, space="DRAM"))
        pom_pool = ctx.enter_context(tc.tile_pool(name="psum", bufs=8, space="PSUM"))

        nc.gpsimd.load_library(library_config.mlp)

        # ---- load constants/inputs ----
        xsh = cpool.tile([128, 3, 34, 64], BF16, tag="xsh")
        nc.sync.dma_start(xsh[:], xsh_d[:].rearrange("p (s a b) -> p s a b",
                                                     s=3, a=34))
        womT = cpool.tile([128, KK, 128], BF16, tag="womT")
        nc.sync.dma_start(womT[:], womT_d[:].rearrange("p (t m) -> p t m", t=KK))
        wmnT = cpool.tile([128, KK * 2, 128], BF16, tag="wmnT")
        nc.sync.dma_start(wmnT[:], wmnT_d[:].rearrange("p (t m) -> p t m", t=KK * 2))
        biasoff = cpool.tile([128, 1], F32, tag="biasoff")
        nc.sync.dma_start(biasoff[:], biasoff_d[:])
        biasmsk = cpool.tile([128, 1], F32, tag="biasmsk")
        nc.sync.dma_start(biasmsk[:], biasmsk_d[:])
        baset = cpool.tile([128, N], F32, tag="baset")
        nc.sync.dma_start(baset[:], baset_d[:])

        idx_t = cpool.tile([128, N], I16, tag="idx")
        ct = cpool.tile([128, 4, N], BF16, tag="coef")
        wrap16 = cpool.tile([16, KK, 128], I16, tag="wrap16")
        wrap = cpool.tile([128, KK, 128], I16, tag="wrap")
        ctd = dpool.tile([KK, 4 * N], BF16, tag="ctd")

        maskx = [9 + i if i <= 22 else 31 for i in range(32)]
        maskm = [18 + i if i <= 13 else 31 for i in range(32)]
        AL = mybir.AluOpType
        AF = mybir.ActivationFunctionType

        # ---- offset/mask conv + scalar pipeline ----
        for c in range(NPCH):
            vct = ppool.tile([128, PCH], F32, tag="vc")
            for cc in range(PCH // CH):
                pom = pom_pool.tile([128, CH], F32, tag="ps")
                for t in range(KK):
                    ky, kx = t // 3, t % 3
                    r0 = 8 * (c * (PCH // CH) + cc) + t // 3
                    rhs = xsh[:, kx, r0: r0 + 8, :]
                    nc.tensor.matmul(pom[:], womT[:, t, :], rhs,
                                     start=(t == 0), stop=(t == KK - 1))
                nc.vector.tensor_scalar(vct[:, cc * CH:(cc + 1) * CH], pom[:],
                                        -16.0, 16.0, AL.max, AL.min)
            sl = slice(c * PCH, (c + 1) * PCH)
            vc = vct[:]
            if debug:
                nc.sync.dma_start(dbg_conv_d[:, sl], vct[:])
            th = ppool.tile([128, PCH], F32, tag="th")
            nc.scalar.activation(th[:], vc, AF.Tanh, bias=biasmsk[:], scale=0.5)
            thal = ppool.tile([128, PCH], F32, tag="thal")
            nc.vector.stream_shuffle(thal[:], th[:], maskm)
            t_ = ppool.tile([128, PCH], F32, tag="t_")
            nc.vector.scalar_tensor_tensor(t_[:], vc, biasoff[:], baset[:, sl],
                                           AL.add, AL.add)
            f0i = ppool.tile([128, PCH], I16, tag="f0i")
            nc.vector.tensor_scalar(f0i[:], t_[:], FLOOR_DELTA, None, AL.add)
            f0 = ppool.tile([128, PCH], F32, tag="f0")
            nc.scalar.copy(f0[:], f0i[:])
            fr = ppool.tile([128, PCH], F32, tag="fr")
            nc.vector.tensor_tensor(fr[:], t_[:], f0[:], AL.subtract)
            fxal = ppool.tile([128, PCH], F32, tag="fxal")
            nc.vector.stream_shuffle(fxal[:], fr[:], maskx)
            f0xal = ppool.tile([128, PCH], F32, tag="f0xal")
            nc.vector.stream_shuffle(f0xal[:], f0[:], maskx)
            # idx written at transposed positions tau(q) = 128*(q%16) + q//16
            # (within each 2048 block; here per 1024-chunk: cols 32c + ...)
            iap = idx_t[:]
            idx_dst = bass.AP(iap.tensor, iap.offset + 64 * c,
                              [iap.ap[0], [1, 64], [128, 16]])
            nc.vector.scalar_tensor_tensor(idx_dst, f0[:], float(WC), f0xal[:],
                                           AL.mult, AL.add)
            my1 = ppool.tile([128, PCH], F32, tag="my1")
            nc.vector.scalar_tensor_tensor(my1[:], thal[:], 1.0, fr[:],
                                           AL.add, AL.mult)
            my0 = ppool.tile([128, PCH], F32, tag="my0")
            nc.vector.scalar_tensor_tensor(my0[:], thal[:], 1.0, my1[:],
                                           AL.add, AL.subtract)
            omfx = ppool.tile([128, PCH], F32, tag="omfx")
            nc.vector.tensor_scalar(omfx[:], fxal[:], -1.0, 1.0, AL.mult, AL.add)
            nc.vector.tensor_tensor(ct[:, 0, sl], my0[:], omfx[:], AL.mult)
            nc.vector.tensor_tensor(ct[:, 1, sl], my1[:], omfx[:], AL.mult)
            nc.vector.tensor_tensor(ct[:, 2, sl], my0[:], fxal[:], AL.mult)
            nc.vector.tensor_tensor(ct[:, 3, sl], my1[:], fxal[:], AL.mult)
            # wrap this chunk's idx cols into dma_gather layout + stage coefs
            for kk in range(KK):
                src_row = idx_t[kk: kk + 1, :]
                src = bass.AP(src_row.tensor, src_row.offset + 64 * c,
                              [src_row.ap[0], [128, 16], [1, 64]])
                nc.sync.dma_start(wrap16[:, kk, 64 * c:64 * c + 64], src)
            for g in range(8):
                nc.sync.dma_start(wrap[16 * g:16 * (g + 1), :, 64 * c:64 * c + 64],
                                  wrap16[:, :, 64 * c:64 * c + 64])
            dstc = bass.AP(ctd[:].tensor, ctd[:].offset + c * PCH,
                           [[4 * N, KK], [N, 4], [1, PCH]])
            nc.sync.dma_start(dstc, ct[0:KK, :, sl])


        if debug:
            nc.sync.dma_start(dbg_idx_d[:], idx_t[:])
            nc.sync.dma_start(dbg_ct_d[:], ct[:].rearrange("p a b -> p (a b)"))

        # ---- per (half, tap): gather + coef replication + combine + matmul ----
        # Split in two sample-halves so gathers overlap the second pipe chunk.
        # Main-matmul accumulation is kk-outer: all 8 (m, chunk) PSUM banks
        # stay open across the kk loop so PE work rides along the gathers.
        N2 = N // 2
        cap = canvas_d[:]
        cview = bass.AP(cap.tensor, cap.offset, [[ES // 2, HC * WC], [1, ES]])
        pstiles = []
        for _i in range(8):
            pst = pom_pool.tile([128, CH], F32, tag="ps")
            pstiles.append(pst)
        for hf in range(2):
            hsl = slice(hf * N2, (hf + 1) * N2)
            for kk in range(KK):
                G = gpool.tile([128, 4, N2], BF16, tag="G")
                nc.gpsimd.dma_gather(G[:], cview, wrap[:, kk, 64 * hf:64 * hf + 64],
                                     N2, N2, ES, elem_step=ES // 2, transpose=True,
                                     single_packet=False)
                crep = rpool.tile([128, 4, N2], BF16, tag="crep")
                if kk < N_SHUF:
                    for k4 in range(4):
                        nc.vector.stream_shuffle(crep[:, k4, :], ct[:, k4, hsl],
                                                 [kk] * 32)
                elif kk < N_SHUF + N_POOL:
                    p0 = rpool.tile([1, 4 * N2], BF16, tag="p0stage")
                    nc.sync.dma_start(p0[:], ct[kk: kk + 1, :, hsl])
                    nc.gpsimd.partition_broadcast(
                        crep[:].rearrange("p a b -> p (a b)"), p0[:])
                else:
                    dap = ctd[kk: kk + 1, :]
                    src = bass.AP(dap.tensor, dap.offset + hf * N2,
                                  [[0, 128], [N, 4], [1, N2]])
                    nc.sync.dma_start(crep[:], src)
                nc.vector.tensor_tensor(G[:], G[:], crep[:], AL.mult)
                nc.vector.tensor_tensor(G[:, 0:2, :], G[:, 0:2, :], G[:, 2:4, :],
                                        AL.add)
                s = spool.tile([128, N2], BF16, tag="s")
                nc.vector.tensor_tensor(s[:], G[:, 0, :], G[:, 1, :], AL.add)
                if debug:
                    nc.sync.dma_start(dbg_s_d[kk, :, hsl], s[:])
                for m in range(2):
                    for cc in range(N2 // CH):
                        nc.tensor.matmul(pstiles[m * NCHUNK + hf * (N2 // CH) + cc][:],
                                         wmnT[:, kk * 2 + m, :],
                                         s[:, cc * CH:(cc + 1) * CH],
                                         start=(kk == 0), stop=(kk == KK - 1))

        for m in range(2):
            for c in range(NCHUNK):
                ob = opool.tile([128, CH], F32, tag="ob")
                nc.scalar.copy(ob[:], pstiles[m * NCHUNK + c][:])
                nc.sync.dma_start(out_d[m, :, c * CH:(c + 1) * CH], ob[:])

    nc.compile()
    return nc


def _prep_core_inputs(x, offset_w, offset_b, mod_w, mod_b, weight, b, h):
    """Host-side layout prep for core (b, h). Pure reshaping/padding/casting."""
    f32 = np.float32
    # xsh: 3 horizontally-shifted copies (kx = 0,1,2) of rows h*32-1..h*32+32,
    # zero-padded, 64 wide, bf16 -> contiguous conv rhs slices
    xpad = np.zeros((128, 34, 66), f32)
    r0 = h * HALF - 1
    for i in range(34):
        r = r0 + i
        if 0 <= r < H:
            xpad[:, i, 1:65] = x[b, :, r, :]
    xsh = np.stack([xpad[:, :, kx:kx + 64] for kx in range(3)], axis=1)
    xsh = np.ascontiguousarray(xsh).astype(BF)                        # [128,3,34,64]
    # canvas: padded channels-last row-pair canvas (per batch), bf16
    xcl = np.ascontiguousarray(x[b].transpose(1, 2, 0)).astype(BF)    # [64,64,128]
    padded = np.zeros((101, WC, 128), BF)
    padded[PADC:PADC + H, PADC:PADC + W, :] = xcl
    canvas = np.concatenate([padded[:-1], padded[1:]], axis=2)        # [100,104,256]
    canvas = canvas.reshape(HC * WC, ES // 2)
    canvas = np.ascontiguousarray(np.vstack([canvas, np.zeros((1, ES // 2), BF)]))
    # womT: lhsT per tap, quadrant-replicated 27 output rows
    wsel = np.zeros((32, CIN, K, K), f32)
    for j in range(9):
        wsel[j] = offset_w[2 * j]
        wsel[9 + j] = offset_w[2 * j + 1]
        wsel[18 + j] = mod_w[j]
    womT = np.zeros((128, KK, 128), f32)
    for t in range(KK):
        blk = wsel[:, :, t // 3, t % 3].T                             # [CIN, 32]
        for q in range(4):
            womT[:, t, 32 * q:32 * q + 32] = blk
    womT = womT.astype(BF)
    # wmnT: lhsT per (tap, m-half) in bf16
    wmnT = np.zeros((128, KK * 2, 128), BF)
    for t in range(KK):
        wt = weight[:, :, t // 3, t % 3]                              # [COUT, CIN]
        for m in range(2):
            wmnT[:, t * 2 + m, :] = wt[m * 128:(m + 1) * 128, :].T.astype(BF)
    # bias vectors, quadrant-replicated
    boff = np.zeros((32, 1), f32)
    bmsk = np.zeros((32, 1), f32)
    for j in range(9):
        boff[j, 0] = offset_b[2 * j]
        boff[9 + j, 0] = offset_b[2 * j + 1]
        bmsk[18 + j, 0] = 0.5 * mod_b[j]
    biasoff = np.tile(boff, (4, 1))
    biasmsk = np.tile(bmsk, (4, 1))
    # base table: sampling-grid origin + canvas shift, quadrant-replicated
    pp = np.arange(N)
    rr = pp // W
    ww = pp % W
    bt = np.zeros((32, N), f32)
    for j in range(9):
        bt[j] = h * HALF + rr + (j // 3) - 1 + PADC
        bt[9 + j] = ww + (j % 3) - 1 + PADC
    baset = np.tile(bt, (4, 1))
    return {
        "xsh": xsh.reshape(128, 3 * 34 * 64),
        "canvas": canvas,
        "womT": womT.reshape(128, KK * 128),
        "wmnT": wmnT.reshape(128, KK * 2 * 128),
        "biasoff": biasoff,
        "biasmsk": biasmsk,
        "baset": baset,
    }


def make_in_maps(x, offset_w, offset_b, mod_w, mod_b, weight):
    return [
        _prep_core_inputs(x, offset_w, offset_b, mod_w, mod_b, weight,
                          core // 2, core % 2)
        for core in range(NCORES)
    ]


def get_program(debug=False):
    key = ("nc", debug)
    if key not in _cache:
        _cache[key] = _build_program(debug)
    return _cache[key]


def assemble_output(results):
    out = np.zeros((B, COUT, H, W), np.float32)
    for core in range(NCORES):
        b, h = core // 2, core % 2
        r = results[core]["out"]                                     # [2,128,N]
        out[b, :, h * HALF:(h + 1) * HALF, :] = r.reshape(COUT, HALF, W)
    return out


def kernel(x, offset_w, offset_b, mod_w, mod_b, weight):
    x = np.asarray(x, np.float32)
    offset_w = np.asarray(offset_w, np.float32)
    offset_b = np.asarray(offset_b, np.float32)
    mod_w = np.asarray(mod_w, np.float32)
    mod_b = np.asarray(mod_b, np.float32)
    weight = np.asarray(weight, np.float32)
    nc = get_program()
    in_maps = make_in_maps(x, offset_w, offset_b, mod_w, mod_b, weight)
    res = run_bass_kernel_spmd(nc, in_maps, list(range(NCORES)))
    return assemble_output(res.results)


# revision 46
# speedup vs baseline: 1.2826x; 1.2826x over previous
"""Deformable Conv2d (B=4, Cin=128, Cout=256, H=W=64, K=3, s=1, p=1) on 8 trn2 cores.

Sharding: core = 2*b + h  (batch b, row-half h: rows h*32 .. h*32+31).
Per-core pipeline:
  - offset/mask 3x3 conv on PE (bf16, contiguous pre-shifted inputs),
    outputs quadrant-replicated in SBUF
  - DVE/ACT scalar pipeline -> bilinear corner coefs (bf16) + int16 gather idx
  - dma_gather from a host-built padded channels-last row-pair canvas in HBM:
    one 1KB element = 2x2 corner patch x 128 channels (bf16)
  - coef replication across partitions: stride-0 SBUF->SBUF DMA broadcast
    (+ one DVE stream_shuffle to balance engines)
  - bf16 combine (coef x corners) on DVE
  - main matmul: 9 taps x 2 Cout tiles, bf16, PSUM-accumulated -> fp32 out
"""
import numpy as np
import ml_dtypes
from contextlib import ExitStack

import concourse.bacc as bacc
import concourse.bass as bass
import concourse.mybir as mybir
import concourse.tile as tile
from concourse import library_config
from concourse.bass_utils import run_bass_kernel_spmd

B, CIN, COUT, H, W, K = 4, 128, 256, 64, 64, 3
KK = K * K
NCORES = 8
HALF = H // 2            # 32 rows per core
N = HALF * W             # 2048 output positions per core
CH = 512                 # matmul chunk size (PSUM bank limit, fp32)
NCHUNK = N // CH
PCH = 1024               # scalar-pipeline chunk size
NPCH = N // PCH
PADC = 18                # canvas padding (covers reference clip of +-16 + tap + bilinear)
HC = 100                 # canvas row-pairs  (y' = y + PADC, y in [-18, 81])
WC = 104                 # canvas cols (x' = x + PADC)
ES = 512                 # gather elem size in bf16 elements (1KB): 2x2 patch x 128ch
F32 = mybir.dt.float32
BF16 = mybir.dt.bfloat16
I16 = mybir.dt.int16
BF = ml_dtypes.bfloat16

_cache = {}

# f32->int16 convert on HW DVE rounds-to-nearest-even -> floor(t) = int(t-0.5).
# CoreSim models truncation -> floor(t) = int(t). Sim checks set this to 0.0.
FLOOR_DELTA = -0.5
# How many taps use DVE stream_shuffle for coef replication (rest use
# stride-0 SBUF->SBUF DMA broadcast). Balances DVE vs DMA engine time.
N_SHUF = 2
N_POOL = 4
# per-(half, tap) replication engine: V=DVE shuffle, P=Pool broadcast, D=DMA
REPL = "VVPPPPDDD" "VVPPPPDDD"  # u = hf*9 + kk


def _build_program(debug=False):
    nc = bacc.Bacc("TRN2", target_bir_lowering=False, debug=False,
                   enable_asserts=False, num_devices=NCORES)
    if debug:
        dbg_conv_d = nc.dram_tensor("dbg_conv", [128, N], F32,
                                    kind="ExternalOutput")
        dbg_idx_d = nc.dram_tensor("dbg_idx", [128, N], I16, kind="ExternalOutput")
        dbg_ct_d = nc.dram_tensor("dbg_ct", [128, 4 * N], BF16,
                                  kind="ExternalOutput")
        dbg_s_d = nc.dram_tensor("dbg_s", [KK, 128, N], BF16,
                                 kind="ExternalOutput")
        dbg_g_d = nc.dram_tensor("dbg_g", [128, 4 * N], BF16,
                                 kind="ExternalOutput")
    xsh_d = nc.dram_tensor("xsh", [128, 3 * 34 * 64], BF16, kind="ExternalInput")
    canvas_d = nc.dram_tensor("canvas", [HC * WC + 1, ES // 2], BF16,
                              kind="ExternalInput")
    womT_d = nc.dram_tensor("womT", [128, KK * 128], BF16, kind="ExternalInput")
    wmnT_d = nc.dram_tensor("wmnT", [128, KK * 2 * 128], BF16, kind="ExternalInput")
    biasoff_d = nc.dram_tensor("biasoff", [128, 1], F32, kind="ExternalInput")
    biasmsk_d = nc.dram_tensor("biasmsk", [128, 1], F32, kind="ExternalInput")
    baset_d = nc.dram_tensor("baset", [128, N], F32, kind="ExternalInput")
    out_d = nc.dram_tensor("out", [2, 128, N], F32, kind="ExternalOutput")

    with tile.TileContext(nc) as tc, ExitStack() as ctx:
        cpool = ctx.enter_context(tc.tile_pool(name="const", bufs=1))
        ppool = ctx.enter_context(tc.tile_pool(name="pipe", bufs=1))
        gpool = ctx.enter_context(tc.tile_pool(name="gath", bufs=4))
        rpool = ctx.enter_context(tc.tile_pool(name="crep", bufs=5))
        spool = ctx.enter_context(tc.tile_pool(name="samp", bufs=4))
        opool = ctx.enter_context(tc.tile_pool(name="outp", bufs=2))
        dpool = ctx.enter_context(tc.tile_pool(name="dram", bufs=1, space="DRAM"))
        pom_pool = ctx.enter_context(tc.tile_pool(name="psum", bufs=8, space="PSUM"))

        nc.gpsimd.load_library(library_config.mlp)

        # ---- load constants/inputs (conv deps first) ----
        xshr = xsh_d[:].rearrange("p (s a b) -> p s a b", s=3, a=34)
        xsh = cpool.tile([128, 3, 34, 64], BF16, tag="xsh")
        nc.sync.dma_start(xsh[:, :, 0:18, :], xshr[:, :, 0:18, :])
        womT = cpool.tile([128, KK, 128], BF16, tag="womT")
        nc.sync.dma_start(womT[:], womT_d[:].rearrange("p (t m) -> p t m", t=KK))
        biasoff = cpool.tile([128, 1], F32, tag="biasoff")
        nc.sync.dma_start(biasoff[:], biasoff_d[:])
        biasmsk = cpool.tile([128, 1], F32, tag="biasmsk")
        nc.sync.dma_start(biasmsk[:], biasmsk_d[:])
        baset = cpool.tile([128, N], F32, tag="baset")
        nc.sync.dma_start(baset[:, 0:PCH], baset_d[:, 0:PCH])
        nc.sync.dma_start(xsh[:, :, 18:34, :], xshr[:, :, 18:34, :])
        nc.sync.dma_start(baset[:, PCH:N], baset_d[:, PCH:N])
        wmnT = cpool.tile([128, KK * 2, 128], BF16, tag="wmnT")
        nc.sync.dma_start(wmnT[:], wmnT_d[:].rearrange("p (t m) -> p t m", t=KK * 2))

        idx_t = cpool.tile([128, N], I16, tag="idx")
        ct = cpool.tile([128, 4, N], BF16, tag="coef")
        wrap16 = cpool.tile([16, KK, 128], I16, tag="wrap16")
        wrap = cpool.tile([128, KK, 128], I16, tag="wrap")
        ctd = dpool.tile([KK, 4 * N], BF16, tag="ctd")

        maskx = [9 + i if i <= 22 else 31 for i in range(32)]
        maskm = [18 + i if i <= 13 else 31 for i in range(32)]
        AL = mybir.AluOpType
        AF = mybir.ActivationFunctionType

        # ---- offset/mask conv + scalar pipeline ----
        for c in range(NPCH):
            vct = ppool.tile([128, PCH], F32, tag="vc")
            for cc in range(PCH // CH):
                pom = pom_pool.tile([128, CH], F32, tag="ps")
                for t in range(KK):
                    ky, kx = t // 3, t % 3
                    r0 = 8 * (c * (PCH // CH) + cc) + t // 3
                    rhs = xsh[:, kx, r0: r0 + 8, :]
                    nc.tensor.matmul(pom[:], womT[:, t, :], rhs,
                                     start=(t == 0), stop=(t == KK - 1))
                nc.vector.tensor_scalar(vct[:, cc * CH:(cc + 1) * CH], pom[:],
                                        -16.0, 16.0, AL.max, AL.min)
            sl = slice(c * PCH, (c + 1) * PCH)
            vc = vct[:]
            if debug:
                nc.sync.dma_start(dbg_conv_d[:, sl], vct[:])
            th = ppool.tile([128, PCH], F32, tag="th")
            nc.scalar.activation(th[:], vc, AF.Tanh, bias=biasmsk[:], scale=0.5)
            thal = ppool.tile([128, PCH], F32, tag="thal")
            nc.vector.stream_shuffle(thal[:], th[:], maskm)
            t_ = ppool.tile([128, PCH], F32, tag="t_")
            nc.vector.scalar_tensor_tensor(t_[:], vc, biasoff[:], baset[:, sl],
                                           AL.add, AL.add)
            f0i = ppool.tile([128, PCH], I16, tag="f0i")
            nc.vector.tensor_scalar(f0i[:], t_[:], FLOOR_DELTA, None, AL.add)
            f0 = ppool.tile([128, PCH], F32, tag="f0")
            nc.scalar.copy(f0[:], f0i[:])
            fr = ppool.tile([128, PCH], F32, tag="fr")
            nc.vector.tensor_tensor(fr[:], t_[:], f0[:], AL.subtract)
            fxal = ppool.tile([128, PCH], F32, tag="fxal")
            nc.vector.stream_shuffle(fxal[:], fr[:], maskx)
            f0xal = ppool.tile([128, PCH], F32, tag="f0xal")
            nc.vector.stream_shuffle(f0xal[:], f0[:], maskx)
            # idx written at transposed positions tau(q) = 128*(q%16) + q//16
            # (within each 2048 block; here per 1024-chunk: cols 32c + ...)
            iap = idx_t[:]
            idx_dst = bass.AP(iap.tensor, iap.offset + 64 * c,
                              [iap.ap[0], [1, 64], [128, 16]])
            nc.vector.scalar_tensor_tensor(idx_dst, f0[:], float(WC), f0xal[:],
                                           AL.mult, AL.add)
            my1 = ppool.tile([128, PCH], F32, tag="my1")
            nc.vector.scalar_tensor_tensor(my1[:], thal[:], 1.0, fr[:],
                                           AL.add, AL.mult)
            my0 = ppool.tile([128, PCH], F32, tag="my0")
            nc.vector.scalar_tensor_tensor(my0[:], thal[:], 1.0, my1[:],
                                           AL.add, AL.subtract)
            omfx = ppool.tile([128, PCH], F32, tag="omfx")
            nc.scalar.activation(omfx[:], fxal[:], AF.Copy, bias=1.0, scale=-1.0)
            nc.vector.tensor_tensor(ct[:, 0, sl], my0[:], omfx[:], AL.mult)
            nc.vector.tensor_tensor(ct[:, 1, sl], my1[:], omfx[:], AL.mult)
            nc.vector.tensor_tensor(ct[:, 2, sl], my0[:], fxal[:], AL.mult)
            nc.vector.tensor_tensor(ct[:, 3, sl], my1[:], fxal[:], AL.mult)
            # wrap this chunk's idx cols into dma_gather layout + stage coefs
            for kk in range(KK):
                src_row = idx_t[kk: kk + 1, :]
                src = bass.AP(src_row.tensor, src_row.offset + 64 * c,
                              [src_row.ap[0], [128, 16], [1, 64]])
                nc.sync.dma_start(wrap16[:, kk, 64 * c:64 * c + 64], src)
            for g in range(8):
                nc.sync.dma_start(wrap[16 * g:16 * (g + 1), :, 64 * c:64 * c + 64],
                                  wrap16[:, :, 64 * c:64 * c + 64])
            dstc = bass.AP(ctd[:].tensor, ctd[:].offset + c * PCH,
                           [[4 * N, KK], [N, 4], [1, PCH]])
            nc.sync.dma_start(dstc, ct[0:KK, :, sl])


        if debug:
            nc.sync.dma_start(dbg_idx_d[:], idx_t[:])
            nc.sync.dma_start(dbg_ct_d[:], ct[:].rearrange("p a b -> p (a b)"))

        # ---- per (half, tap): gather + coef replication + combine + matmul ----
        # Split in two sample-halves so gathers overlap the second pipe chunk.
        # Main-matmul accumulation is kk-outer: all 8 (m, chunk) PSUM banks
        # stay open across the kk loop so PE work rides along the gathers.
        N2 = N // 2
        cap = canvas_d[:]
        cview = bass.AP(cap.tensor, cap.offset, [[ES // 2, HC * WC], [1, ES]])
        pstiles = []
        for _i in range(8):
            pst = pom_pool.tile([128, CH], F32, tag="ps")
            pstiles.append(pst)
        for hf in range(2):
            hsl = slice(hf * N2, (hf + 1) * N2)
            for kk in range(KK):
                G = gpool.tile([128, 4, N2], BF16, tag="G")
                nc.gpsimd.dma_gather(G[:], cview, wrap[:, kk, 64 * hf:64 * hf + 64],
                                     N2, N2, ES, elem_step=ES // 2, transpose=True,
                                     single_packet=False)
                crep = rpool.tile([128, 4, N2], BF16, tag="crep")
                mode = REPL[hf * KK + kk]
                if mode == "V":
                    for k4 in range(4):
                        nc.vector.stream_shuffle(crep[:, k4, :], ct[:, k4, hsl],
                                                 [kk] * 32)
                elif mode == "P":
                    p0 = opool.tile([1, 2 * N2], F32, tag="p0stage")
                    nc.sync.dma_start(p0[:], ct[kk: kk + 1, :, hsl].bitcast(F32))
                    nc.gpsimd.partition_broadcast(
                        crep[:].rearrange("p a b -> p (a b)").bitcast(F32), p0[:])
                else:
                    dap = ctd[kk: kk + 1, :]
                    src = bass.AP(dap.tensor, dap.offset + hf * N2,
                                  [[0, 128], [N, 4], [1, N2]])
                    nc.sync.dma_start(crep[:], src)
                nc.vector.tensor_tensor(G[:], G[:], crep[:], AL.mult)
                nc.vector.tensor_tensor(G[:, 0:2, :], G[:, 0:2, :], G[:, 2:4, :],
                                        AL.add)
                if debug:
                    s = spool.tile([128, N2], BF16, tag="s")
                    nc.vector.tensor_tensor(s[:], G[:, 0, :], G[:, 1, :], AL.add)
                    nc.sync.dma_start(dbg_s_d[kk, :, hsl], s[:])
                for m in range(2):
                    for cc in range(N2 // CH):
                        for prt in range(2):
                            nc.tensor.matmul(
                                pstiles[m * NCHUNK + hf * (N2 // CH) + cc][:],
                                wmnT[:, kk * 2 + m, :],
                                G[:, prt, cc * CH:(cc + 1) * CH],
                                start=(kk == 0 and prt == 0),
                                stop=(kk == KK - 1 and prt == 1))

        for m in range(2):
            for c in range(NCHUNK):
                ob = opool.tile([128, CH], F32, tag="ob")
                if (m * NCHUNK + c) % 2 == 0:
                    nc.vector.tensor_copy(ob[:], pstiles[m * NCHUNK + c][:])
                else:
                    nc.scalar.copy(ob[:], pstiles[m * NCHUNK + c][:])
                nc.sync.dma_start(out_d[m, :, c * CH:(c + 1) * CH], ob[:])

    nc.compile()
    return nc


def _prep_core_inputs(x, offset_w, offset_b, mod_w, mod_b, weight, b, h):
    """Host-side layout prep for core (b, h). Pure reshaping/padding/casting."""
    f32 = np.float32
    # xsh: 3 horizontally-shifted copies (kx = 0,1,2) of rows h*32-1..h*32+32,
    # zero-padded, 64 wide, bf16 -> contiguous conv rhs slices
    xpad = np.zeros((128, 34, 66), f32)
    r0 = h * HALF - 1
    for i in range(34):
        r = r0 + i
        if 0 <= r < H:
            xpad[:, i, 1:65] = x[b, :, r, :]
    xsh = np.stack([xpad[:, :, kx:kx + 64] for kx in range(3)], axis=1)
    xsh = np.ascontiguousarray(xsh).astype(BF)                        # [128,3,34,64]
    # canvas: padded channels-last row-pair canvas (per batch), bf16
    xcl = np.ascontiguousarray(x[b].transpose(1, 2, 0)).astype(BF)    # [64,64,128]
    padded = np.zeros((101, WC, 128), BF)
    padded[PADC:PADC + H, PADC:PADC + W, :] = xcl
    canvas = np.concatenate([padded[:-1], padded[1:]], axis=2)        # [100,104,256]
    canvas = canvas.reshape(HC * WC, ES // 2)
    canvas = np.ascontiguousarray(np.vstack([canvas, np.zeros((1, ES // 2), BF)]))
    # womT: lhsT per tap, quadrant-replicated 27 output rows
    wsel = np.zeros((32, CIN, K, K), f32)
    for j in range(9):
        wsel[j] = offset_w[2 * j]
        wsel[9 + j] = offset_w[2 * j + 1]
        wsel[18 + j] = mod_w[j]
    womT = np.zeros((128, KK, 128), f32)
    for t in range(KK):
        blk = wsel[:, :, t // 3, t % 3].T                             # [CIN, 32]
        for q in range(4):
            womT[:, t, 32 * q:32 * q + 32] = blk
    womT = womT.astype(BF)
    # wmnT: lhsT per (tap, m-half) in bf16
    wmnT = np.zeros((128, KK * 2, 128), BF)
    for t in range(KK):
        wt = weight[:, :, t // 3, t % 3]                              # [COUT, CIN]
        for m in range(2):
            wmnT[:, t * 2 + m, :] = wt[m * 128:(m + 1) * 128, :].T.astype(BF)
    # bias vectors, quadrant-replicated
    boff = np.zeros((32, 1), f32)
    bmsk = np.zeros((32, 1), f32)
    for j in range(9):
        boff[j, 0] = offset_b[2 * j]
        boff[9 + j, 0] = offset_b[2 * j + 1]
        bmsk[18 + j, 0] = 0.5 * mod_b[j]
    biasoff = np.tile(boff, (4, 1))
    biasmsk = np.tile(bmsk, (4, 1))
    # base table: sampling-grid origin + canvas shift, quadrant-replicated
    pp = np.arange(N)
    rr = pp // W
    ww = pp % W
    bt = np.zeros((32, N), f32)
    for j in range(9):
        bt[j] = h * HALF + rr + (j // 3) - 1 + PADC
        bt[9 + j] = ww + (j % 3) - 1 + PADC
    baset = np.tile(bt, (4, 1))
    return {
        "xsh": xsh.reshape(128, 3 * 34 * 64),
        "canvas": canvas,
        "womT": womT.reshape(128, KK * 128),
        "wmnT": wmnT.reshape(128, KK * 2 * 128),
        "biasoff": biasoff,
        "biasmsk": biasmsk,
        "baset": baset,
    }


def make_in_maps(x, offset_w, offset_b, mod_w, mod_b, weight):
    return [
        _prep_core_inputs(x, offset_w, offset_b, mod_w, mod_b, weight,
                          core // 2, core % 2)
        for core in range(NCORES)
    ]


def get_program(debug=False):
    key = ("nc", debug)
    if key not in _cache:
        _cache[key] = _build_program(debug)
    return _cache[key]


def assemble_output(results):
    out = np.zeros((B, COUT, H, W), np.float32)
    for core in range(NCORES):
        b, h = core // 2, core % 2
        r = results[core]["out"]                                     # [2,128,N]
        out[b, :, h * HALF:(h + 1) * HALF, :] = r.reshape(COUT, HALF, W)
    return out


def kernel(x, offset_w, offset_b, mod_w, mod_b, weight):
    x = np.asarray(x, np.float32)
    offset_w = np.asarray(offset_w, np.float32)
    offset_b = np.asarray(offset_b, np.float32)
    mod_w = np.asarray(mod_w, np.float32)
    mod_b = np.asarray(mod_b, np.float32)
    weight = np.asarray(weight, np.float32)
    nc = get_program()
    in_maps = make_in_maps(x, offset_w, offset_b, mod_w, mod_b, weight)
    res = run_bass_kernel_spmd(nc, in_maps, list(range(NCORES)))
    return assemble_output(res.results)
